# revision 38
# baseline (speedup 1.0000x reference)
"""AsymAttentionLayer Trainium2 kernel — data-parallel over B on 8 NeuronCores.

Reference computation (per batch element b, NUM_G=32, g=32, num_p=8, dim=512,
H=8, E=64):
  stage1: attention within groups of g=32 tokens (seq=(ng,p), pos=gg)
  stage2: attention across groups (seq=(gg,p), pos=ng)
  7 linears of [tokens,512]x[512,512].

Device-side layout: feature-major activations xT [dim, token]; token order
t = ng*256 + p*32 + gg (stage-1 sequences contiguous).  All matmuls bf16 with
f32 PSUM accumulation.  Host pre-transposes/casts inputs and un-permutes the
output, so the device graph does zero transposes of x.
"""

import os
import sys

import numpy as np

sys.path.insert(0, "/opt/trn_rl_repo")

NUM_G = 32
G = 32
NUM_P = 8
B = 8
D = 512
H = 8
E = 64
T = NUM_G * G * NUM_P  # 8192 tokens per core
CH = 512  # tokens per chunk
NCHUNK = T // CH  # 16
NSEQ = CH // G  # 16 sequences per chunk

# Attention weight-load grouping: how many [64x32] q-blocks (scores) /
# [32x64] v-blocks (AV) share one LDWEIGHTS.  1 = one LDW per matmul
# (self-loading, original behavior).
GROUP_S = int(os.environ.get("KB_GROUP_S", "4"))
GROUP_AV = int(os.environ.get("KB_GROUP_AV", "4"))
EVAC_SPLIT = os.environ.get("KB_EVAC_SPLIT", "1") == "1"

_GRAPH = None
LAST_EXEC_TIME_NS = None
LAST_TRACE = None


def _build_graph():
    import concourse.bass as bass
    from concourse import bacc, mybir
    from concourse.tile import TileContext

    f32 = mybir.dt.float32
    bf16 = mybir.dt.bfloat16
    AF = mybir.ActivationFunctionType
    ALU = mybir.AluOpType
    AX = mybir.AxisListType

    nc = bacc.Bacc()

    # group-LDW integrity tracking (verified post-finalize): mm name ->
    # (group ldw name, row0, row1); ldw name -> (row0, row1)
    grp_mm = {}
    grp_ldw = {}
    # tagged MMs whose legalizer-inserted per-MM LDW had sync attached and
    # was therefore kept (safe fallback: it reloads the same data)
    kept_ldw = {}

    def _ins(r):
        return getattr(r, "ins", r)

    # The tile legalizer inserts a per-matmul InstLdweights unconditionally
    # (it ignores InstMatmult.ldweights).  Filter its output: drop the
    # redundant per-MM loads for grouped matmuls — their group LDW already
    # loads the same array region.  Only sync-free inserted LDWs are
    # dropped (observed: legalizer LDWs carry no sync/descendants); any
    # sync-carrying one is kept, which is always safe since it reloads
    # identical data.
    import concourse.tile as _tile_mod

    def _ldw_region(ins):
        tp = ins.tile_position or (0, 0)
        ts = ins.tile_size or (128, 128)
        return (tp[0], tp[0] + ts[0], tp[1], tp[1] + ts[1])

    def _overlap(a, b):
        return a[0] < b[1] and b[0] < a[1] and a[2] < b[3] and b[2] < a[3]

    def _legalize_filter(orig):
        def wrapped(ordered, nc_, *a, **kw):
            out = orig(ordered, nc_, *a, **kw)
            n_drop = n_keep = 0
            for bb in list(out.keys()):
                insts = out[bb]
                # inserted-LDW name -> the tagged MM that follows it
                ins_ldw_mm = {}
                for i, ins in enumerate(insts[:-1]):
                    nxt = insts[i + 1]
                    if (
                        isinstance(ins, mybir.InstLdweights)
                        and ins.name not in grp_ldw
                        and isinstance(nxt, mybir.InstMatmult)
                        and nxt.name in grp_mm
                    ):
                        ins_ldw_mm[ins.name] = nxt.name
                if not ins_ldw_mm:
                    continue
                # all LDW events in order: (idx, name, region)
                ldw_events = [
                    (i, ins.name,
                     grp_ldw.get(ins.name) or _ldw_region(ins))
                    for i, ins in enumerate(insts)
                    if isinstance(ins, mybir.InstLdweights)
                ]
                pos = {ins.name: i for i, ins in enumerate(insts)}
                # decide per tagged MM: droppable iff its group LDW precedes
                # it and no foreign LDW overlapping the MM's weight region
                # sits in between (its own inserted load is exempt)
                drop = set()
                for i, ins in enumerate(insts):
                    if not (isinstance(ins, mybir.InstMatmult) and ins.name in grp_mm):
                        continue
                    g, reg = grp_mm[ins.name]
                    gp = pos.get(g)
                    if gp is None or gp > i:
                        continue
                    clean = True
                    for li, ln, lreg in ldw_events:
                        if li <= gp or li >= i:
                            continue
                        if ins_ldw_mm.get(ln) == ins.name:
                            continue
                        if _overlap(lreg, reg):
                            clean = False
                            break
                    if clean:
                        drop.add(ins.name)
                n_drop += len(drop)
                new = []
                i, n = 0, len(insts)
                while i < n:
                    ins = insts[i]
                    mmn = ins_ldw_mm.get(getattr(ins, "name", None))
                    if mmn is not None:
                        si = ins.sync_info
                        syncfree = si is None or (not si.on_wait and not si.on_update)
                        if mmn in drop and syncfree:
                            i += 1  # drop the redundant load
                            continue
                        kept_ldw[mmn] = ins.name
                        n_keep += 1
                    new.append(ins)
                    i += 1
                out[bb] = new
            print(
                f"[kernel] group-LDW: {n_drop} per-MM loads dropped, "
                f"{n_keep} kept",
                file=sys.stderr,
            )
            return out

        return wrapped

    xT_d = nc.declare_dram_parameter("xT", [D, T], bf16, isOutput=False)
    w_d = {}
    for name in ("wq1", "wk1", "wv1", "wq2", "wk2", "wv2", "wo"):
        w_d[name] = nc.declare_dram_parameter(name, [D, D], bf16, isOutput=False)
    b_d = {}
    for name in ("bq1", "bk1", "bv1", "bq2", "bk2", "bv2", "bo"):
        b_d[name] = nc.declare_dram_parameter(name, [128, 4], f32, isOutput=False)
    out_d = nc.declare_dram_parameter("out", [D, T], f32, isOutput=True)

    _orig_legalize = _tile_mod.tile_legalize
    _tile_mod.tile_legalize = _legalize_filter(_orig_legalize)
    with TileContext(nc) as tc:
        with (
            tc.tile_pool(name="wpool", bufs=1) as wpool,
            tc.tile_pool(name="bpool", bufs=1) as bpool,
            tc.tile_pool(name="y1pool", bufs=1) as y1pool,
            tc.tile_pool(name="sbx", bufs=3) as sbx,
            tc.tile_pool(name="sbqk", bufs=2) as sbqk,
            tc.tile_pool(name="sbv", bufs=2) as sbv,
            tc.tile_pool(name="sba", bufs=2) as sba,
            tc.tile_pool(name="sbo", bufs=2) as sbo,
            tc.tile_pool(name="pp", bufs=4, space="PSUM") as pp,
            tc.tile_pool(name="ppy", bufs=1, space="PSUM") as ppy,
        ):
            # ---- weights / biases resident in SBUF ----
            # Stage-1 weights (and the first x chunks, DMA'd later) are
            # issued first so the first q-linear can start ASAP.
            wt = {}
            bt = {}

            def load_w(name):
                tiles = []
                for k in range(4):
                    t_ = wpool.tile([128, D], bf16, tag=f"{name}_{k}", name=f"w_{name}_{k}")
                    nc.sync.dma_start(out=t_, in_=w_d[name][128 * k : 128 * (k + 1), :])
                    tiles.append(t_)
                wt[name] = tiles

            def load_b(name):
                t_ = bpool.tile([128, 4], f32, tag=name, name=f"b_{name}")
                nc.sync.dma_start(out=t_, in_=b_d[name][:, :])
                bt[name] = t_

            for name in ("wq1", "wk1", "wv1"):
                load_w(name)
            for name in ("bq1", "bk1", "bv1"):
                load_b(name)

            # stage-1 output, feature-major [D, T] as 4 tiles [128, T]
            y1 = [y1pool.tile([128, T], bf16, tag=f"y1_{r}", name=f"y1_{r}") for r in range(4)]

            def lin_m(w_tiles, bias_tile, rhs_aps, out_tiles, m, on_vector=False):
                """One m-tile of a feature-major linear: out[m] = bias +
                (W^T @ x)[128m:128m+128, :]."""
                ps = pp.tile([128, CH], f32, tag="ps", name="ps_lin")
                for k in range(4):
                    nc.tensor.matmul(
                        ps,
                        lhsT=w_tiles[k][:, 128 * m : 128 * (m + 1)],
                        rhs=rhs_aps[k],
                        start=(k == 0),
                        stop=(k == 3),
                    )
                if on_vector:
                    nc.vector.tensor_scalar_add(
                        out_tiles[m], ps, bias_tile[:, m : m + 1]
                    )
                else:
                    nc.scalar.activation(
                        out=out_tiles[m],
                        in_=ps,
                        func=AF.Identity,
                        bias=bias_tile[:, m : m + 1],
                    )

            def vlin_j(w_tiles, lhsT_aps, out_tiles, j):
                """One token-group of the v-linear: out[j] = [128 tokens,
                512 dims] (token-major), no bias."""
                ps = pp.tile([128, D], f32, tag="ps", name="ps_vlin")
                for k in range(4):
                    nc.tensor.matmul(
                        ps,
                        lhsT=lhsT_aps[j][k],
                        rhs=w_tiles[k],
                        start=(k == 0),
                        stop=(k == 3),
                    )
                nc.vector.tensor_copy(out_tiles[j], ps)

            def scores_quarter(qt, kt, ps_s, m):
                """Scores for head-pair m: 32 of the chunk's 128 problems.

                PSUM bank index always equals the PE row-tile index so that
                concurrently-running row tiles never write the same bank.
                scores placement: bank=h%2 (= row tile 64*(h%2)), strip=sl%4,
                colblk=(h//2)*4 + sl//4.
                """

                def s_mm(m, s, par, c, ldw_name):
                    # h = 2m+par, sl = 4s+c; identical placement to the
                    # ungrouped formulas (st=32c, cb=32(4m+s), rb=64par).
                    r = nc.tensor.matmul(
                        ps_s[par][32 * c : 32 * c + 32,
                                  32 * (4 * m + s) : 32 * (4 * m + s) + 32],
                        lhsT=qt[m][64 * par : 64 * par + 64,
                                   128 * s + 32 * c : 128 * s + 32 * c + 32],
                        rhs=kt[m][64 * par : 64 * par + 64,
                                  32 * (4 * s + c) : 32 * (4 * s + c) + 32],
                        start=True,
                        stop=True,
                        tile_position=(64 * par, 32 * c),
                    )
                    if ldw_name is not None:
                        grp_mm[_ins(r).name] = (
                            ldw_name,
                            (64 * par, 64 * par + 64, 32 * c, 32 * c + 32),
                        )

                if GROUP_S == 1:
                    for s in range(4):
                        for c in range(4):
                            for par in range(2):
                                s_mm(m, s, par, c, None)
                elif GROUP_S == 8:
                    for s in range(4):
                        lw = nc.tensor.ldweights(
                            qt[m][:, 128 * s : 128 * (s + 1)],
                            tile_position=(0, 0),
                        )
                        grp_ldw[_ins(lw).name] = (0, 128, 0, 128)
                        for par in range(2):
                            for c in range(4):
                                s_mm(m, s, par, c, _ins(lw).name)
                else:  # GROUP_S == 4: paired half-array loads, then MMs
                    for s in range(4):
                        lws = []
                        for par in range(2):
                            lw = nc.tensor.ldweights(
                                qt[m][64 * par : 64 * par + 64,
                                      128 * s : 128 * (s + 1)],
                                tile_position=(64 * par, 0),
                            )
                            grp_ldw[_ins(lw).name] = (
                                64 * par, 64 * par + 64, 0, 128,
                            )
                            lws.append(_ins(lw).name)
                        for par in range(2):
                            for c in range(4):
                                s_mm(m, s, par, c, lws[par])

            def softmax_emit(ps_s):
                """Softmax over s (free dim), batched 64 problems per op;
                returns transposed normalized-A tiles for the AV matmuls."""
                a_f = [sba.tile([128, CH], f32, tag=f"a{i}", name=f"a_f{i}") for i in range(2)]
                sums = sba.tile([128, 32], f32, tag="sums")
                rs = sba.tile([128, 32], f32, tag="rs")
                for sb in range(2):
                    nc.scalar.activation(out=a_f[sb], in_=ps_s[sb], func=AF.Exp)
                    nc.vector.tensor_reduce(
                        out=sums[:, 16 * sb : 16 * sb + 16],
                        in_=a_f[sb].rearrange("p (j s) -> p j s", s=32),
                        axis=AX.X,
                        op=ALU.add,
                    )
                nc.vector.reciprocal(rs, sums)
                a_n = [sba.tile([128, CH], bf16, tag=f"an{i}", name=f"a_n{i}") for i in range(2)]
                a_t = [sba.tile([128, CH], bf16, tag=f"at{i}", name=f"a_t{i}") for i in range(2)]
                for sb in range(2):
                    rs_sl = rs[:, 16 * sb : 16 * sb + 16]
                    rs_b = bass.AP(
                        tensor=rs_sl.tensor,
                        offset=rs_sl.offset,
                        ap=[*rs_sl.ap, [0, 32]],
                    )
                    nc.vector.tensor_mul(
                        a_n[sb].rearrange("p (j s) -> p j s", s=32),
                        a_f[sb].rearrange("p (j s) -> p j s", s=32),
                        rs_b,
                    )
                    nc.vector.transpose(a_t[sb], a_n[sb])
                return a_t

            def av_quarter(vt, a_t, ys, j):
                # --- AV: yT[e, l] blocks.  ys is one 4-bank tile; bank
                # (col 512c) = row tile 32c holds the 32 problems with
                # sl%4 == c; within a bank: partition rows 64*(h%2) = head
                # parity, col block 32*((h//2)*4 + sl//4).
                # Quarter j covers seqs 4j..4j+3 (vt[j]) over all heads.

                def av_mm(j, hh, c, par, ldw_name):
                    # h = 2hh+par, sl = 4j+c
                    cb = 512 * c + 32 * (4 * hh + j)
                    r = nc.tensor.matmul(
                        ys[64 * par : 64 * par + 64, cb : cb + 32],
                        lhsT=vt[j][32 * c : 32 * c + 32,
                                   128 * hh + 64 * par : 128 * hh + 64 * par + 64],
                        rhs=a_t[par][32 * c : 32 * c + 32,
                                     32 * (4 * hh + j) : 32 * (4 * hh + j) + 32],
                        start=True,
                        stop=True,
                        tile_position=(32 * c, 64 * par),
                    )
                    if ldw_name is not None:
                        grp_mm[_ins(r).name] = (
                            ldw_name,
                            (32 * c, 32 * c + 32, 64 * par, 64 * par + 64),
                        )

                if GROUP_AV == 1:
                    for hh in range(4):
                        for c in range(4):
                            for par in range(2):
                                av_mm(j, hh, c, par, None)
                elif GROUP_AV == 8:
                    for hh in range(4):
                        lw = nc.tensor.ldweights(
                            vt[j][:, 128 * hh : 128 * (hh + 1)],
                            tile_position=(0, 0),
                        )
                        grp_ldw[_ins(lw).name] = (0, 128, 0, 128)
                        for c in range(4):
                            for par in range(2):
                                av_mm(j, hh, c, par, _ins(lw).name)
                elif GROUP_AV == 4:
                    for hh in range(4):
                        lws = []
                        for q2 in range(2):
                            lw = nc.tensor.ldweights(
                                vt[j][64 * q2 : 64 * q2 + 64,
                                      128 * hh : 128 * (hh + 1)],
                                tile_position=(64 * q2, 0),
                            )
                            grp_ldw[_ins(lw).name] = (
                                64 * q2, 64 * q2 + 64, 0, 128,
                            )
                            lws.append(_ins(lw).name)
                        for q2 in range(2):
                            for c in (2 * q2, 2 * q2 + 1):
                                for par in range(2):
                                    av_mm(j, hh, c, par, lws[q2])
                else:  # GROUP_AV == 2: one 32-row strip per LDW
                    for hh in range(4):
                        for c in range(4):
                            lw = nc.tensor.ldweights(
                                vt[j][32 * c : 32 * c + 32,
                                      128 * hh : 128 * (hh + 1)],
                                tile_position=(32 * c, 0),
                            )
                            grp_ldw[_ins(lw).name] = (
                                32 * c, 32 * c + 32, 0, 128,
                            )
                            for par in range(2):
                                av_mm(j, hh, c, par, _ins(lw).name)

            def av_evacs(ys, tmp, vbias_tile):
                """Evacuate the whole 4-bank AV psum in 4 contiguous ops:
                op hh reads [j, r, g] (strides 32, 512, 1) and writes tmp[hh]
                cols 0..511 sequentially (tmp col = 32*sl + g, sl = 4j+r)."""
                ysv = ys.rearrange(
                    "p (r hh j g) -> p hh j r g", r=4, hh=4, j=4, g=32
                )
                for hh in range(4):
                    src = ysv[:, hh]
                    dstv = tmp[hh].rearrange("p (j r g) -> p j r g", j=4, r=4)
                    if EVAC_SPLIT and hh % 2 == 1:
                        nc.vector.tensor_scalar_add(
                            dstv, src, vbias_tile[:, hh : hh + 1]
                        )
                    else:
                        nc.scalar.activation(
                            out=dstv,
                            in_=src,
                            func=AF.Identity,
                            bias=vbias_tile[:, hh : hh + 1],
                        )

            # y1 cols use stage-2 token order: col2 = gg*256 + p*32 + ng, so
            # stage-2 linears read contiguous slices.  Stage-1 chunks first
            # evacuate PSUM contiguously (t1-local order) into tmp tiles,
            # then the idle GpSimd engine scatters tmp into y1 (the ng<->gg
            # transpose with its 2-byte-granule writes runs off the
            # critical scalar/vector engines).
            y1sc = [
                y1[k].rearrange("p (gg pp ng) -> p ng pp gg", gg=32, pp=8, ng=32)
                for k in range(4)
            ]

            def produce_qk(i):
                """q/k linears for chunk i (i<16: stage-1, else stage-2) with
                the chunk's scores quarters interleaved per m-tile, so the PE
                alternates dense linear streams and small-matmul bursts."""
                qt = [sbqk.tile([128, CH], bf16, tag=f"qt{m}", name=f"qt{m}") for m in range(4)]
                kt = [sbqk.tile([128, CH], bf16, tag=f"kt{m}", name=f"kt{m}") for m in range(4)]
                vt = [sbv.tile([128, D], bf16, tag=f"vt{j}", name=f"vt{j}") for j in range(4)]
                ps_s = [pp.tile([128, CH], f32, tag="ps", name="ps_s") for _ in range(2)]
                if i < NCHUNK:
                    c = i
                    cols = slice(CH * c, CH * (c + 1))
                    xc = []
                    for k in range(4):
                        t_ = sbx.tile([128, CH], bf16, tag=f"xc{k}", name=f"xc{k}")
                        nc.sync.dma_start(
                            out=t_, in_=xT_d[128 * k : 128 * (k + 1), cols]
                        )
                        xc.append(t_)
                    rhs_aps = [x[:, :] for x in xc]
                    lhsT_aps = [
                        [xc[k][:, 128 * j : 128 * (j + 1)] for k in range(4)]
                        for j in range(4)
                    ]
                    wq, bq, wk, bk, wv, vb = "wq1", "bq1", "wk1", "bk1", "wv1", "bv1"
                    stage2 = False
                else:
                    c2 = i - NCHUNK
                    cols = slice(CH * c2, CH * (c2 + 1))
                    rhs_aps = [y1[k][:, cols] for k in range(4)]
                    lhsT_aps = [
                        [
                            y1[k][:, CH * c2 + 128 * j : CH * c2 + 128 * (j + 1)]
                            for k in range(4)
                        ]
                        for j in range(4)
                    ]
                    wq, bq, wk, bk, wv, vb = "wq2", "bq2", "wk2", "bk2", "wv2", "bv2"
                    stage2 = True
                # tmp receives the chunk's AV output in t-local order
                # (col = 32*sl + g); for stage-2 this IS y2 (consumed by the
                # out-linear), for stage-1 it is scattered into y1 by gpsimd
                tmp = [sbo.tile([128, CH], bf16, tag=f"y2_{r}", name=f"y2_{r}") for r in range(4)]
                # scores quarter m is emitted one m-tile late so its q/k
                # evacuations (scalar q / vector k) have a full linear
                # m-tile of slack to complete before the PE reaches it
                for m in range(4):
                    lin_m(wt[wq], bt[bq], rhs_aps, qt, m)
                    lin_m(wt[wk], bt[bk], rhs_aps, kt, m, on_vector=True)
                    if m > 0:
                        scores_quarter(qt, kt, ps_s, m - 1)
                scores_quarter(qt, kt, ps_s, 3)
                return dict(qt=qt, kt=kt, vt=vt, ps_s=ps_s, lhsT_aps=lhsT_aps,
                            wv=wv, vb=bt[vb], tmp=tmp, stage2=stage2, c=i,
                            a_t=None, ys=None)

            def produce_v_j(st_, j):
                vlin_j(wt[st_["wv"]], st_["lhsT_aps"], st_["vt"], j)

            def av_q(st_, j):
                if st_["ys"] is None:
                    st_["ys"] = ppy.tile([128, 4 * CH], f32, tag="ys", name="ps_y4")
                av_quarter(st_["vt"], st_["a_t"], st_["ys"], j)

            def evac_chunk(st_):
                av_evacs(st_["ys"], st_["tmp"], st_["vb"])
                if not st_["stage2"]:
                    # ng<->gg scatter into stage-2-ordered y1 on GpSimd:
                    # tmp col = 256a + 32p + gg  ->  y1 col = gg*256 + p*32
                    # + (2c+a)
                    c = st_["c"]
                    for k in range(4):
                        nc.gpsimd.tensor_copy(
                            y1sc[k][:, 2 * c : 2 * c + 2, :, :],
                            st_["tmp"][k].rearrange(
                                "p (a pp gg) -> p a pp gg", a=2, pp=8
                            ),
                        )

            def out_linear(c2, y2t):
                for m in range(4):
                    ps = pp.tile([128, CH], f32, tag="ps", name="ps_lin")
                    for k in range(4):
                        nc.tensor.matmul(
                            ps,
                            lhsT=wt["wo"][k][:, 128 * m : 128 * (m + 1)],
                            rhs=y2t[k],
                            start=(k == 0),
                            stop=(k == 3),
                        )
                    os_ = sbo.tile([128, CH], f32, tag=f"os{m}", name=f"os{m}")
                    nc.scalar.activation(
                        out=os_, in_=ps, func=AF.Identity, bias=bt["bo"][:, m : m + 1]
                    )
                    nc.sync.dma_start(
                        out=out_d[128 * m : 128 * (m + 1), CH * c2 : CH * (c2 + 1)],
                        in_=os_,
                    )

            # ---- software pipeline over 32 steps (16 per stage): each step
            # emits [qk+scores(i+1) interleaved per m, outlin(i-1), v(i+1)
            # interleaved with av(i) per j, evacs(i)].  The interleave keeps
            # the PE's low-activity (small-matmul) stretches short so the
            # HAM clock gate stays at full rate.  Exception: stage-2 linears
            # read y1, so produce_qk(16) must follow av_evacs(15).
            NSTEP = 2 * NCHUNK
            cur = produce_qk(0)
            # remaining (stage-2 / out) weights load behind the critical path
            for name in ("wq2", "wk2", "wv2", "wo"):
                load_w(name)
            for name in ("bq2", "bk2", "bv2", "bo"):
                load_b(name)
            cur["a_t"] = softmax_emit(cur["ps_s"])
            for j in range(4):
                produce_v_j(cur, j)
            pend_out = None
            for i in range(NSTEP):
                boundary = i == NCHUNK - 1
                last = i == NSTEP - 1
                nxt = None
                if not last and not boundary:
                    nxt = produce_qk(i + 1)
                if pend_out is not None:
                    out_linear(*pend_out)
                    pend_out = None
                if nxt is not None:
                    nxt["a_t"] = softmax_emit(nxt["ps_s"])
                    for j in range(4):
                        produce_v_j(nxt, j)
                        av_q(cur, j)
                else:
                    for j in range(4):
                        av_q(cur, j)
                evac_chunk(cur)
                if boundary:
                    nxt = produce_qk(i + 1)
                    nxt["a_t"] = softmax_emit(nxt["ps_s"])
                    for j in range(4):
                        produce_v_j(nxt, j)
                if cur["stage2"]:
                    pend_out = (i - NCHUNK, cur["tmp"])
                cur = nxt
            out_linear(*pend_out)
    _tile_mod.tile_legalize = _orig_legalize
    nc.finalize()
    _verify_groups(nc, grp_mm, grp_ldw, kept_ldw)
    return nc


def _verify_groups(nc, grp_mm, grp_ldw, kept_ldw):
    """Walk the final instruction stream and assert that every grouped
    matmul's most recent region-overlapping LDWEIGHTS is exactly the load
    it expects (its group load, or its own kept per-MM load).  A foreign
    weight load landing inside a group would silently corrupt the PE
    array state, so fail hard."""
    from concourse import mybir

    if not grp_mm:
        return

    def ldw_region(ins):
        if ins.name in grp_ldw:
            return grp_ldw[ins.name]
        tp = ins.tile_position or (0, 0)
        ts = ins.tile_size or (128, 128)
        return (tp[0], tp[0] + ts[0], tp[1], tp[1] + ts[1])

    ldws = []  # (name, region)
    checked = 0
    for bb in nc.m.functions[0].blocks:
        for ins in bb.instructions:
            if isinstance(ins, mybir.InstLdweights):
                ldws.append((ins.name, ldw_region(ins)))
            elif isinstance(ins, mybir.InstMatmult) and ins.name in grp_mm:
                want, reg = grp_mm[ins.name]
                want = kept_ldw.get(ins.name, want)
                got = None
                for n, lreg in reversed(ldws):
                    if (lreg[0] < reg[1] and reg[0] < lreg[1]
                            and lreg[2] < reg[3] and reg[2] < lreg[3]):
                        got = n
                        break
                if got != want:
                    raise RuntimeError(
                        f"group-LDW violation: matmul {ins.name} region "
                        f"{reg} expects weights from {want} but last "
                        f"overlapping LDW is {got}"
                    )
                checked += 1
    if checked != len(grp_mm):
        raise RuntimeError(
            f"group-LDW verify: saw {checked} of {len(grp_mm)} grouped matmuls"
        )


def _get_graph():
    global _GRAPH
    if _GRAPH is None:
        _GRAPH = _build_graph()
    return _GRAPH


def _host_pack(x, q1_w, q1_b, k1_w, k1_b, v1_w, v1_b, q2_w, q2_b, k2_w, k2_b,
               v2_w, v2_b, out_w, out_b):
    import ml_dtypes

    bf = ml_dtypes.bfloat16
    scale = 1.0 / np.sqrt(E)

    def wT(w, s=1.0):
        return np.ascontiguousarray((w * s).astype(np.float32).T).astype(bf)

    def bia(b, s=1.0):
        return np.ascontiguousarray((b * s).astype(np.float32).reshape(4, 128).T)

    common = {
        "wq1": wT(q1_w, scale), "wk1": wT(k1_w), "wv1": wT(v1_w),
        "wq2": wT(q2_w, scale), "wk2": wT(k2_w), "wv2": wT(v2_w),
        "wo": wT(out_w),
        "bq1": bia(q1_b, scale), "bk1": bia(k1_b), "bv1": bia(v1_b),
        "bq2": bia(q2_b, scale), "bk2": bia(k2_b), "bv2": bia(v2_b),
        "bo": bia(out_b),
    }
    in_maps = []
    for b in range(B):
        # x[b]: [1024(ch=ng*32+gg), 8(p), 512] -> token order t = ng*256+p*32+gg
        xb = np.asarray(x[b]).reshape(NUM_G, G, NUM_P, D)
        xb = xb.transpose(0, 2, 1, 3).reshape(T, D)
        xT = np.ascontiguousarray(xb.T).astype(bf)
        m = dict(common)
        m["xT"] = xT
        in_maps.append(m)
    return in_maps


def _host_unpack(results):
    # device out: [512, 8192] f32, cols packed:
    # col(ng,gg,p) = (gg//2)*512 + (gg%2)*256 + p*32 + ng
    ng_, gg_, p_ = np.meshgrid(
        np.arange(NUM_G), np.arange(G), np.arange(NUM_P), indexing="ij"
    )
    idx = (gg_ // 2) * 512 + (gg_ % 2) * 256 + p_ * 32 + ng_
    out = np.empty((B, NUM_G * G, NUM_P, D), dtype=np.float32)
    for b in range(B):
        y = results[b]["out"].T  # [8192, 512]
        out[b] = y[idx].reshape(NUM_G * G, NUM_P, D)
    return out


def kernel(**inputs):
    global LAST_EXEC_TIME_NS, LAST_TRACE
    from concourse.bass_utils import run_bass_kernel_spmd

    nc = _get_graph()
    in_maps = _host_pack(**inputs)
    trace = os.environ.get("KBENCH_TRACE") == "1"
    res = run_bass_kernel_spmd(nc, in_maps, list(range(8)), trace=trace)
    LAST_EXEC_TIME_NS = res.exec_time_ns
    it = res.instructions_and_trace
    LAST_TRACE = it[1] if it else None
    return _host_unpack(res.results)

